# revision 37
# baseline (speedup 1.0000x reference)
"""Bass/Tile TRN2 kernel: multi-head attention with a local (sliding-window)
causal mask, window = 128, fp16 compute with fp32 PSUM accumulation.

Problem: x[2, 4096, 1024], 16 heads x 64 dims, out = attn(x) @ Wo^T.

Sharding (8 cores): core c handles batch b = c // 4 and the 4 heads
h in [4*(c%4), 4*(c%4)+4). Each core computes its q/k/v projections
(256 output dims), local attention, and a partial output projection
[4096, 1024] over its 256 contraction dims. The host sums the 4 partials
per batch and adds the (softmax + 1e-9) rank-1 correction plus biases.

Key layout choice: scores are computed TRANSPOSED (S^T[k, q] per
128x128 block, keys on partitions) so the PV matmul's stationary
operand is exactly the exp'd score block -- no P^T transposes needed.
The per-head denominator comes from a 1.0 column appended to v.
The only transpose left is ctx -> ctx^T for the output projection
(2 PE transposes per block vs the 10 a direct-scores layout needs).

The attention loop is software-pipelined 3 deep (QK(i) | PV(i-1) |
out-proj(i-2)) so the PE never waits on the exp/mask/normalize chain
of the block it just produced.

Device layouts per core:
  qT/kT  [dk_on_partitions, seq]   (kT padded with a zero j-block in front)
  v      [j_on_partitions, 4*(64+2)]  (per key-block; col 64 of each head
                                       group is 1.0 -> PV matmul emits the
                                       softmax denominator for free)
  S^T    [k_on_partitions, 4 heads * 2 jblocks * 128 q] in 2 psum banks
  ctx    [q_on_partitions, 4*65] psum -> normalized fp16 -> xbar transpose
"""

import numpy as np
from contextlib import ExitStack

D_MODEL = 1024
SEQ = 4096
BATCH = 2
D_K = 64
O = 256            # head dims per core (4 heads x 64)
WIN = 128
SCALE = 0.125      # 1/sqrt(64)
N_CORES = 8
NB = SEQ // 128    # 32 query/key blocks
NST = SEQ // 512   # 8 projection column tiles

_CACHE = {}


def _build_program():
    import concourse.tile as tile
    from concourse import bacc, mybir

    f16 = mybir.dt.float16
    f32 = mybir.dt.float32
    AF = mybir.ActivationFunctionType

    nc = bacc.Bacc("TRN2", target_bir_lowering=False, debug=False,
                   num_devices=N_CORES)

    xt_d = nc.dram_tensor("xt", [NST, 128, 8, 512], f16,
                          kind="ExternalInput").ap()
    wq_d = nc.dram_tensor("wq", [128, 8, O], f16, kind="ExternalInput").ap()
    wk_d = nc.dram_tensor("wk", [128, 8, O], f16, kind="ExternalInput").ap()
    wv_d = nc.dram_tensor("wv", [128, 8, O], f16, kind="ExternalInput").ap()
    wo_d = nc.dram_tensor("wo", [128, 2, D_MODEL], f16,
                          kind="ExternalInput").ap()
    mi_d = nc.dram_tensor("maskin", [128, 1024], f16, kind="ExternalInput").ap()
    m0_d = nc.dram_tensor("mask0", [128, 1024], f16, kind="ExternalInput").ap()
    out_d = nc.dram_tensor("out", [SEQ, D_MODEL], f16, kind="ExternalOutput").ap()

    with tile.TileContext(nc) as tc, ExitStack() as ctx:
        consts = ctx.enter_context(tc.tile_pool(name="consts", bufs=1))
        store = ctx.enter_context(tc.tile_pool(name="store", bufs=1))
        xts = ctx.enter_context(tc.tile_pool(name="xts", bufs=3))
        pmrs = ctx.enter_context(tc.tile_pool(name="pmrs", bufs=2))
        pms = ctx.enter_context(tc.tile_pool(name="pms", bufs=3))
        cns = ctx.enter_context(tc.tile_pool(name="cns", bufs=3))
        cts = ctx.enter_context(tc.tile_pool(name="cts", bufs=3))
        recs = ctx.enter_context(tc.tile_pool(name="recs", bufs=4))
        outs = ctx.enter_context(tc.tile_pool(name="outs", bufs=4))
        ps4 = ctx.enter_context(tc.tile_pool(name="ps4", bufs=2, space="PSUM"))
        pp = ctx.enter_context(tc.tile_pool(name="pp", bufs=2, space="PSUM"))
        pctx = ctx.enter_context(tc.tile_pool(name="pctx", bufs=1, space="PSUM"))
        ptp = ctx.enter_context(tc.tile_pool(name="ptp", bufs=1, space="PSUM"))

        # ---- constants ----
        # wq + the first x tile gate the first matmul: stream them chunked
        # across BOTH HWDGE queues so the PE starts after ~2 chunks instead
        # of after 1.5 MB of serial DMA. Later-needed consts follow on the
        # scalar queue.
        wq_sb = consts.tile([128, 8, O], f16)
        wk_sb = consts.tile([128, 8, O], f16)
        wv_sb = consts.tile([128, 8, O], f16)
        xt0 = xts.tile([128, 8, 512], f16, tag="xt")
        nc.sync.dma_start(out=wq_sb[:, 0:4], in_=wq_d[:, 0:4])
        nc.sync.dma_start(out=xt0[:, 0:2], in_=xt_d[0][:, 0:2])
        nc.sync.dma_start(out=xt0[:, 4:6], in_=xt_d[0][:, 4:6])
        nc.scalar.dma_start(out=wq_sb[:, 4:8], in_=wq_d[:, 4:8])
        nc.scalar.dma_start(out=xt0[:, 2:4], in_=xt_d[0][:, 2:4])
        nc.scalar.dma_start(out=xt0[:, 6:8], in_=xt_d[0][:, 6:8])
        nc.scalar.dma_start(out=wk_sb, in_=wk_d)
        nc.scalar.dma_start(out=wv_sb, in_=wv_d)
        wo_sb = consts.tile([128, 2, D_MODEL], f16)
        nc.scalar.dma_start(out=wo_sb, in_=wo_d)
        mi_sb = consts.tile([128, 1024], f16)
        m0_sb = consts.tile([128, 1024], f16)
        nc.scalar.dma_start(out=mi_sb, in_=mi_d)
        nc.scalar.dma_start(out=m0_sb, in_=m0_d)
        ident = consts.tile([128, 128], f16)
        from concourse.masks import make_identity
        make_identity(nc, ident)

        qT = store.tile([128, 2, SEQ], f16)
        kT = store.tile([128, 2, 128 + SEQ], f16)   # zero j-block in front
        v = store.tile([128, NB, 4 * (D_K + 2)], f16)
        nc.vector.memset(kT[:, :, 0:128], 0.0)
        v4 = v.rearrange("p j (h e) -> p j h e", e=D_K + 2)
        for h in range(4):
            nc.vector.memset(v4[:, :, h, D_K:D_K + 2], 1.0)

        # ---- projections ----
        for st in range(NST):
            s0 = st * 512
            if st == 0:
                xt = xt0
            else:
                xt = xts.tile([128, 8, 512], f16, tag="xt")
                eng = nc.sync if st % 2 else nc.scalar
                eng.dma_start(out=xt, in_=xt_d[st])
            for w_sb, dst, off in ((wq_sb, qT, 0), (wk_sb, kT, 128)):
                ps = ps4.tile([128, 1024], mybir.dt.float32, tag="s4")
                for ot in range(2):
                    for dc in range(8):
                        nc.tensor.matmul(
                            ps[:, ot * 512:(ot + 1) * 512],
                            lhsT=w_sb[:, dc, ot * 128:(ot + 1) * 128],
                            rhs=xt[:, dc, :],
                            start=(dc == 0), stop=(dc == 7))
                nc.scalar.copy(
                    out=dst[:, :, off + s0:off + s0 + 512],
                    in_=ps.rearrange("p (o s) -> p o s", s=512))
            ps = ps4.tile([128, 1024], mybir.dt.float32, tag="s4")
            for ss in range(4):
                for dc in range(8):
                    nc.tensor.matmul(
                        ps[:, ss * 256:(ss + 1) * 256],
                        lhsT=xt[:, dc, ss * 128:(ss + 1) * 128],
                        rhs=wv_sb[:, dc, :],
                        start=(dc == 0), stop=(dc == 7))
            nc.vector.tensor_copy(
                out=v4[:, st * 4:(st + 1) * 4, :, 0:D_K],
                in_=ps.rearrange("p (j h e) -> p j h e", h=4, e=D_K))

        # ---- attention + output projection, 3-stage pipeline ----
        def issue_qk(ib):
            """S^T blocks for all 4 heads: keys on partitions, queries free.
            Bank b holds heads with row-group parity b; within a bank the
            column slot is (h//2)*256 + a*128 for key block a (0 = previous
            block, 1 = same block; kT's zero pad covers ib == 0)."""
            i0 = ib * 128
            st = ps4.tile([128, 1024], mybir.dt.float32, tag="s4",
                          name=f"st_{ib}")
            for h in range(4):
                p0 = (h % 2) * 64
                g = h // 2
                for a in range(2):
                    c0 = (h % 2) * 512 + g * 256 + a * 128
                    nc.tensor.matmul(
                        st[:, c0:c0 + 128],
                        lhsT=kT[p0:p0 + 64, g, i0 + a * 128:i0 + (a + 1) * 128],
                        rhs=qT[p0:p0 + 64, g, i0:i0 + 128],
                        start=True, stop=True)
            pmr = pmrs.tile([128, 1024], f16, tag="pmr")
            nc.scalar.activation(out=pmr, in_=st, func=AF.Exp)
            pm = pms.tile([128, 1024], f16, tag="pm")
            nc.vector.tensor_mul(pm, pmr, m0_sb if ib == 0 else mi_sb)
            return pm

        def issue_pv(ib, pm):
            cps = pctx.tile([128, 4 * (D_K + 1)], mybir.dt.float32, tag="cps")
            for h in range(4):
                c0 = (h % 2) * 512 + (h // 2) * 256
                alist = [a for a in (0, 1) if ib - 1 + a >= 0]
                for idx, a in enumerate(alist):
                    nc.tensor.matmul(
                        cps[:, h * 65:h * 65 + 65],
                        lhsT=pm[:, c0 + a * 128:c0 + (a + 1) * 128],
                        rhs=v[:, ib - 1 + a, h * 66:h * 66 + 65],
                        start=(idx == 0), stop=(idx == len(alist) - 1))
            cn = cns.tile([128, 2, 128], f16, tag="cn")
            rec4 = recs.tile([128, 4], mybir.dt.float32, tag="rec")
            cps4 = cps.rearrange("p (h e) -> p h e", e=D_K + 1)
            nc.vector.reciprocal(
                rec4, cps4[:, :, D_K:D_K + 1].rearrange("p h one -> p (h one)"))
            nc.vector.tensor_mul(
                cn.rearrange("p g (x e) -> p (g x) e", e=D_K),
                cps4[:, :, 0:D_K],
                rec4.unsqueeze(2).broadcast_to([128, 4, D_K]))
            return cn

        def issue_outproj(ib, cn):
            ctp = ptp.tile([128, 256], f16, tag="ptp")
            for cc in range(2):
                nc.tensor.transpose(
                    ctp[:, cc * 128:(cc + 1) * 128], cn[:, cc, :], ident)
            ct = cts.tile([128, 2, 128], f16, tag="ct")
            nc.scalar.copy(out=ct.rearrange("p a i -> p (a i)"), in_=ctp)
            po = [pp.tile([128, 512], mybir.dt.float32, tag="pp",
                          name=f"po_{ib}_{mh}") for mh in range(2)]
            for cc in range(2):
                for mh in range(2):
                    nc.tensor.matmul(
                        po[mh],
                        lhsT=ct[:, cc, :],
                        rhs=wo_sb[:, cc, mh * 512:(mh + 1) * 512],
                        start=(cc == 0), stop=(cc == 1))
            i0 = ib * 128
            ob = outs.tile([128, 1024], f16, tag="ob")
            nc.scalar.copy(out=ob[:, 0:512], in_=po[0])
            nc.vector.tensor_copy(out=ob[:, 512:1024], in_=po[1])
            eng = nc.sync if ib % 2 else nc.scalar
            eng.dma_start(out=out_d[i0:i0 + 128, :], in_=ob)

        pm_q = {}
        cn_q = {}
        for ib in range(NB + 3):
            if ib < NB:
                pm_q[ib] = issue_qk(ib)
            if 0 <= ib - 2 < NB:
                cn_q[ib - 2] = issue_pv(ib - 2, pm_q.pop(ib - 2))
            if ib - 3 >= 0:
                issue_outproj(ib - 3, cn_q.pop(ib - 3))
    nc.compile()
    return nc


def get_program():
    if "nc" not in _CACHE:
        _CACHE["nc"] = _build_program()
    return _CACHE["nc"]


def _masks():
    """Masks in the S^T layout: [key row r, query col c] per 128x128 block.
    Key block a=0 (previous block): allowed iff r >= c; a=1 (same block):
    allowed iff r <= c. Column layout matches the psum slots:
    bank-major [g0a0, g0a1, g1a0, g1a1] x 2 banks."""
    r = np.arange(128)[:, None]
    c = np.arange(128)[None, :]
    lo = (r >= c).astype(np.float16)
    up = (r <= c).astype(np.float16)
    pair = np.concatenate([lo, up], axis=1)
    pair0 = np.concatenate([np.zeros_like(lo), up], axis=1)
    return np.tile(pair, (1, 4)), np.tile(pair0, (1, 4))


def make_in_maps(inputs):
    x = np.asarray(inputs["x"], np.float32)
    Wq = np.asarray(inputs["Wq"], np.float32)
    Wk = np.asarray(inputs["Wk"], np.float32)
    Wv = np.asarray(inputs["Wv"], np.float32)
    Wo = np.asarray(inputs["Wo"], np.float32)
    MI, M0 = _masks()
    in_maps = []
    for core in range(N_CORES):
        b, g = core // 4, core % 4
        sl = slice(g * O, (g + 1) * O)
        xt = np.ascontiguousarray(
            x[b].reshape(NST, 512, 8, 128).transpose(0, 3, 2, 1)
        ).astype(np.float16)
        wtile = lambda w: np.ascontiguousarray(
            w.reshape(-1, 128, w.shape[1]).transpose(1, 0, 2)
        ).astype(np.float16)
        in_maps.append({
            "xt": xt,
            "wq": wtile((Wq[sl] * SCALE).T),
            "wk": wtile(Wk[sl].T),
            "wv": wtile(Wv[sl].T),
            "wo": wtile(Wo[:, sl].T),
            "maskin": MI,
            "mask0": M0,
        })
    return in_maps


def combine(results, inputs):
    """Sum per-core partials and add host-side corrections."""
    x = np.asarray(inputs["x"], np.float32)
    Wv = np.asarray(inputs["Wv"], np.float32)
    Wo = np.asarray(inputs["Wo"], np.float32)
    bv = np.asarray(inputs["bv"], np.float32)
    bo = np.asarray(inputs["bo"], np.float32)
    out = np.zeros((BATCH, SEQ, D_MODEL), np.float32)
    for core in range(N_CORES):
        out[core // 4] += results[core]["out"]
    # reference adds 1e-9 to every attn prob (including masked ones):
    # ctx += 1e-9 * sum_j v[j]  ->  out += 1e-9 * (sum_j v[j]) @ Wo^T
    for b in range(BATCH):
        vs = x[b].sum(axis=0) @ Wv.T + SEQ * bv
        out[b] += (1e-9 * (vs @ Wo.T) + bo)[None, :]
    return out


def run_cores(in_maps, trace=False, **kw):
    from concourse.bass_utils import run_bass_kernel_spmd
    nc = get_program()
    return run_bass_kernel_spmd(nc, in_maps, core_ids=list(range(N_CORES)),
                                trace=trace, **kw)


def kernel(**inputs):
    in_maps = make_in_maps(inputs)
    res = run_cores(in_maps)
    return combine(res.results, inputs)


# revision 39
# speedup vs baseline: 1.0377x; 1.0377x over previous
"""Bass/Tile TRN2 kernel: multi-head attention with a local (sliding-window)
causal mask, window = 128, fp16 compute with fp32 PSUM accumulation.

Problem: x[2, 4096, 1024], 16 heads x 64 dims, out = attn(x) @ Wo^T.

Sharding (8 cores): core c handles batch b = c // 4 and the 4 heads
h in [4*(c%4), 4*(c%4)+4). Each core computes its q/k/v projections
(256 output dims), local attention, and a partial output projection
[4096, 1024] over its 256 contraction dims. The host sums the 4 partials
per batch and adds the (softmax + 1e-9) rank-1 correction plus biases.

Key layout choice: scores are computed TRANSPOSED (S^T[k, q] per
128x128 block, keys on partitions) so the PV matmul's stationary
operand is exactly the exp'd score block -- no P^T transposes needed.
The per-head denominator comes from a 1.0 column appended to v.
The only transpose left is ctx -> ctx^T for the output projection
(2 PE transposes per block vs the 10 a direct-scores layout needs).

The attention loop is software-pipelined 3 deep (QK(i) | PV(i-1) |
out-proj(i-2)) so the PE never waits on the exp/mask/normalize chain
of the block it just produced.

Device layouts per core:
  qT/kT  [dk_on_partitions, seq]   (kT padded with a zero j-block in front)
  v      [j_on_partitions, 4*(64+2)]  (per key-block; col 64 of each head
                                       group is 1.0 -> PV matmul emits the
                                       softmax denominator for free)
  S^T    [k_on_partitions, 4 heads * 2 jblocks * 128 q] in 2 psum banks
  ctx    [q_on_partitions, 4*65] psum -> normalized fp16 -> xbar transpose
"""

import numpy as np
from contextlib import ExitStack

D_MODEL = 1024
SEQ = 4096
BATCH = 2
D_K = 64
O = 256            # head dims per core (4 heads x 64)
WIN = 128
SCALE = 0.125      # 1/sqrt(64)
N_CORES = 8
NB = SEQ // 128    # 32 query/key blocks
NST = SEQ // 512   # 8 projection column tiles

_CACHE = {}


def _build_program():
    import concourse.tile as tile
    from concourse import bacc, mybir

    f16 = mybir.dt.float16
    f32 = mybir.dt.float32
    AF = mybir.ActivationFunctionType

    nc = bacc.Bacc("TRN2", target_bir_lowering=False, debug=False,
                   num_devices=N_CORES)

    xt_d = nc.dram_tensor("xt", [NST, 128, 8, 512], f16,
                          kind="ExternalInput").ap()
    wq_d = nc.dram_tensor("wq", [128, 8, O], f16, kind="ExternalInput").ap()
    wk_d = nc.dram_tensor("wk", [128, 8, O], f16, kind="ExternalInput").ap()
    wv_d = nc.dram_tensor("wv", [128, 8, O], f16, kind="ExternalInput").ap()
    wo_d = nc.dram_tensor("wo", [128, 2, D_MODEL], f16,
                          kind="ExternalInput").ap()
    mi_d = nc.dram_tensor("maskin", [128, 1024], f16, kind="ExternalInput").ap()
    m0_d = nc.dram_tensor("mask0", [128, 1024], f16, kind="ExternalInput").ap()
    out_d = nc.dram_tensor("out", [SEQ, D_MODEL], f16, kind="ExternalOutput").ap()

    with tile.TileContext(nc) as tc, ExitStack() as ctx:
        consts = ctx.enter_context(tc.tile_pool(name="consts", bufs=1))
        store = ctx.enter_context(tc.tile_pool(name="store", bufs=1))
        xts = ctx.enter_context(tc.tile_pool(name="xts", bufs=3))
        pmrs = ctx.enter_context(tc.tile_pool(name="pmrs", bufs=2))
        pms = ctx.enter_context(tc.tile_pool(name="pms", bufs=3))
        cns = ctx.enter_context(tc.tile_pool(name="cns", bufs=3))
        cts = ctx.enter_context(tc.tile_pool(name="cts", bufs=3))
        recs = ctx.enter_context(tc.tile_pool(name="recs", bufs=4))
        outs = ctx.enter_context(tc.tile_pool(name="outs", bufs=4))
        ps4 = ctx.enter_context(tc.tile_pool(name="ps4", bufs=2, space="PSUM"))
        pp = ctx.enter_context(tc.tile_pool(name="pp", bufs=2, space="PSUM"))
        pctx = ctx.enter_context(tc.tile_pool(name="pctx", bufs=1, space="PSUM"))
        ptp = ctx.enter_context(tc.tile_pool(name="ptp", bufs=1, space="PSUM"))

        # ---- constants ----
        # wq + the first x tile gate the first matmul: stream them chunked
        # across BOTH HWDGE queues so the PE starts after ~2 chunks instead
        # of after 1.5 MB of serial DMA. Later-needed consts follow on the
        # scalar queue.
        wq_sb = consts.tile([128, 8, O], f16)
        wk_sb = consts.tile([128, 8, O], f16)
        wv_sb = consts.tile([128, 8, O], f16)
        xt0 = xts.tile([128, 8, 512], f16, tag="xt")
        nc.sync.dma_start(out=wq_sb[:, 0:4], in_=wq_d[:, 0:4])
        nc.sync.dma_start(out=xt0[:, 0:2], in_=xt_d[0][:, 0:2])
        nc.sync.dma_start(out=xt0[:, 4:6], in_=xt_d[0][:, 4:6])
        nc.scalar.dma_start(out=wq_sb[:, 4:8], in_=wq_d[:, 4:8])
        nc.scalar.dma_start(out=xt0[:, 2:4], in_=xt_d[0][:, 2:4])
        nc.scalar.dma_start(out=xt0[:, 6:8], in_=xt_d[0][:, 6:8])
        nc.scalar.dma_start(out=wk_sb, in_=wk_d)
        nc.scalar.dma_start(out=wv_sb, in_=wv_d)
        wo_sb = consts.tile([128, 2, D_MODEL], f16)
        nc.scalar.dma_start(out=wo_sb, in_=wo_d)
        mi_sb = consts.tile([128, 1024], f16)
        m0_sb = consts.tile([128, 1024], f16)
        nc.scalar.dma_start(out=mi_sb, in_=mi_d)
        nc.scalar.dma_start(out=m0_sb, in_=m0_d)
        ident = consts.tile([128, 128], f16)
        from concourse.masks import make_identity
        make_identity(nc, ident)

        qT = store.tile([128, 2, SEQ], f16)
        kT = store.tile([128, 2, 128 + SEQ], f16)   # zero j-block in front
        v = store.tile([128, NB, 4 * (D_K + 2)], f16)
        nc.vector.memset(kT[:, :, 0:128], 0.0)
        v4 = v.rearrange("p j (h e) -> p j h e", e=D_K + 2)
        for h in range(4):
            nc.vector.memset(v4[:, :, h, D_K:D_K + 2], 1.0)

        # ---- projections ----
        for st in range(NST):
            s0 = st * 512
            if st == 0:
                xt = xt0
            else:
                xt = xts.tile([128, 8, 512], f16, tag="xt")
                eng = nc.sync if st % 2 else nc.scalar
                eng.dma_start(out=xt, in_=xt_d[st])
            for w_sb, dst, off in ((wq_sb, qT, 0), (wk_sb, kT, 128)):
                ps = ps4.tile([128, 1024], mybir.dt.float32, tag="s4")
                for ot in range(2):
                    for dc in range(8):
                        nc.tensor.matmul(
                            ps[:, ot * 512:(ot + 1) * 512],
                            lhsT=w_sb[:, dc, ot * 128:(ot + 1) * 128],
                            rhs=xt[:, dc, :],
                            start=(dc == 0), stop=(dc == 7))
                nc.scalar.copy(
                    out=dst[:, :, off + s0:off + s0 + 512],
                    in_=ps.rearrange("p (o s) -> p o s", s=512))
            ps = ps4.tile([128, 1024], mybir.dt.float32, tag="s4")
            for ss in range(4):
                for dc in range(8):
                    nc.tensor.matmul(
                        ps[:, ss * 256:(ss + 1) * 256],
                        lhsT=xt[:, dc, ss * 128:(ss + 1) * 128],
                        rhs=wv_sb[:, dc, :],
                        start=(dc == 0), stop=(dc == 7))
            nc.vector.tensor_copy(
                out=v4[:, st * 4:(st + 1) * 4, :, 0:D_K],
                in_=ps.rearrange("p (j h e) -> p j h e", h=4, e=D_K))

        # ---- attention + output projection, 3-stage pipeline ----
        def issue_qk(ib):
            """S^T blocks for all 4 heads: keys on partitions, queries free.
            Bank b holds heads with row-group parity b; within a bank the
            column slot is (h//2)*256 + a*128 for key block a (0 = previous
            block, 1 = same block; kT's zero pad covers ib == 0)."""
            i0 = ib * 128
            st = ps4.tile([128, 1024], mybir.dt.float32, tag="s4",
                          name=f"st_{ib}")
            for h in range(4):
                p0 = (h % 2) * 64
                g = h // 2
                for a in range(2):
                    c0 = (h % 2) * 512 + g * 256 + a * 128
                    nc.tensor.matmul(
                        st[:, c0:c0 + 128],
                        lhsT=kT[p0:p0 + 64, g, i0 + a * 128:i0 + (a + 1) * 128],
                        rhs=qT[p0:p0 + 64, g, i0:i0 + 128],
                        start=True, stop=True)
            pmr = pmrs.tile([128, 1024], f16, tag="pmr")
            nc.scalar.activation(out=pmr, in_=st, func=AF.Exp)
            pm = pms.tile([128, 1024], f16, tag="pm")
            nc.vector.tensor_mul(pm, pmr, m0_sb if ib == 0 else mi_sb)
            return pm

        def issue_pv(ib, pm):
            cps = pctx.tile([128, 4 * (D_K + 1)], mybir.dt.float32, tag="cps")
            for h in range(4):
                c0 = (h % 2) * 512 + (h // 2) * 256
                alist = [a for a in (0, 1) if ib - 1 + a >= 0]
                for idx, a in enumerate(alist):
                    nc.tensor.matmul(
                        cps[:, h * 65:h * 65 + 65],
                        lhsT=pm[:, c0 + a * 128:c0 + (a + 1) * 128],
                        rhs=v[:, ib - 1 + a, h * 66:h * 66 + 65],
                        start=(idx == 0), stop=(idx == len(alist) - 1))
            cn = cns.tile([128, 2, 128], f16, tag="cn")
            rec4 = recs.tile([128, 4], mybir.dt.float32, tag="rec")
            cps4 = cps.rearrange("p (h e) -> p h e", e=D_K + 1)
            nc.vector.reciprocal(
                rec4, cps4[:, :, D_K:D_K + 1].rearrange("p h one -> p (h one)"))
            nc.vector.tensor_mul(
                cn.rearrange("p g (x e) -> p (g x) e", e=D_K),
                cps4[:, :, 0:D_K],
                rec4.unsqueeze(2).broadcast_to([128, 4, D_K]))
            return cn

        def issue_outproj(ib, cn):
            ctp = ptp.tile([128, 256], f16, tag="ptp")
            for cc in range(2):
                nc.tensor.transpose(
                    ctp[:, cc * 128:(cc + 1) * 128], cn[:, cc, :], ident)
            ct = cts.tile([128, 2, 128], f16, tag="ct")
            nc.scalar.copy(out=ct.rearrange("p a i -> p (a i)"), in_=ctp)
            po = [pp.tile([128, 512], mybir.dt.float32, tag="pp",
                          name=f"po_{ib}_{mh}") for mh in range(2)]
            for cc in range(2):
                for mh in range(2):
                    nc.tensor.matmul(
                        po[mh],
                        lhsT=ct[:, cc, :],
                        rhs=wo_sb[:, cc, mh * 512:(mh + 1) * 512],
                        start=(cc == 0), stop=(cc == 1))
            i0 = ib * 128
            ob = outs.tile([128, 1024], f16, tag="ob")
            nc.scalar.copy(out=ob[:, 0:512], in_=po[0])
            nc.vector.tensor_copy(out=ob[:, 512:1024], in_=po[1])
            nc.sync.dma_start(out=out_d[i0:i0 + 128, :], in_=ob)

        pm_q = {}
        cn_q = {}
        for ib in range(NB + 4):
            if ib < NB:
                pm_q[ib] = issue_qk(ib)
            if 0 <= ib - 2 < NB:
                cn_q[ib - 2] = issue_pv(ib - 2, pm_q.pop(ib - 2))
            if ib - 4 >= 0:
                issue_outproj(ib - 4, cn_q.pop(ib - 4))
    nc.compile()
    return nc


def get_program():
    if "nc" not in _CACHE:
        _CACHE["nc"] = _build_program()
    return _CACHE["nc"]


def _masks():
    """Masks in the S^T layout: [key row r, query col c] per 128x128 block.
    Key block a=0 (previous block): allowed iff r >= c; a=1 (same block):
    allowed iff r <= c. Column layout matches the psum slots:
    bank-major [g0a0, g0a1, g1a0, g1a1] x 2 banks."""
    r = np.arange(128)[:, None]
    c = np.arange(128)[None, :]
    lo = (r >= c).astype(np.float16)
    up = (r <= c).astype(np.float16)
    pair = np.concatenate([lo, up], axis=1)
    pair0 = np.concatenate([np.zeros_like(lo), up], axis=1)
    return np.tile(pair, (1, 4)), np.tile(pair0, (1, 4))


def make_in_maps(inputs):
    x = np.asarray(inputs["x"], np.float32)
    Wq = np.asarray(inputs["Wq"], np.float32)
    Wk = np.asarray(inputs["Wk"], np.float32)
    Wv = np.asarray(inputs["Wv"], np.float32)
    Wo = np.asarray(inputs["Wo"], np.float32)
    MI, M0 = _masks()
    in_maps = []
    for core in range(N_CORES):
        b, g = core // 4, core % 4
        sl = slice(g * O, (g + 1) * O)
        xt = np.ascontiguousarray(
            x[b].reshape(NST, 512, 8, 128).transpose(0, 3, 2, 1)
        ).astype(np.float16)
        wtile = lambda w: np.ascontiguousarray(
            w.reshape(-1, 128, w.shape[1]).transpose(1, 0, 2)
        ).astype(np.float16)
        in_maps.append({
            "xt": xt,
            "wq": wtile((Wq[sl] * SCALE).T),
            "wk": wtile(Wk[sl].T),
            "wv": wtile(Wv[sl].T),
            "wo": wtile(Wo[:, sl].T),
            "maskin": MI,
            "mask0": M0,
        })
    return in_maps


def combine(results, inputs):
    """Sum per-core partials and add host-side corrections."""
    x = np.asarray(inputs["x"], np.float32)
    Wv = np.asarray(inputs["Wv"], np.float32)
    Wo = np.asarray(inputs["Wo"], np.float32)
    bv = np.asarray(inputs["bv"], np.float32)
    bo = np.asarray(inputs["bo"], np.float32)
    out = np.zeros((BATCH, SEQ, D_MODEL), np.float32)
    for core in range(N_CORES):
        out[core // 4] += results[core]["out"]
    # reference adds 1e-9 to every attn prob (including masked ones):
    # ctx += 1e-9 * sum_j v[j]  ->  out += 1e-9 * (sum_j v[j]) @ Wo^T
    for b in range(BATCH):
        vs = x[b].sum(axis=0) @ Wv.T + SEQ * bv
        out[b] += (1e-9 * (vs @ Wo.T) + bo)[None, :]
    return out


def run_cores(in_maps, trace=False, **kw):
    from concourse.bass_utils import run_bass_kernel_spmd
    nc = get_program()
    return run_bass_kernel_spmd(nc, in_maps, core_ids=list(range(N_CORES)),
                                trace=trace, **kw)


def kernel(**inputs):
    in_maps = make_in_maps(inputs)
    res = run_cores(in_maps)
    return combine(res.results, inputs)
